# revision 32
# baseline (speedup 1.0000x reference)
"""Fused AttnBlock kernel for 8 Trainium2 NeuronCores.

Problem: q = LN_head(x1 @ wq + bq), k = LN_head(x2 @ wk + bk), v = x2 @ wv + bv,
out = softmax(q k^T / sqrt(D)) v, with B=4, N=2048, C=1024, H=16, D=64.

Sharding: data-parallel over batch (4) x tensor-parallel over head groups (2).
Each core handles one (batch, head-group) pair fully locally: its 8 heads'
columns of wq/wkv are contiguous slices, so there are no collectives; the host
scatters inputs (pre-transposed + cast to bf16) and gathers/transposes outputs.

Per-core dataflow:
  - host pre-transposes x1/x2 so the contraction dim (C) lands on partitions;
    all big matmuls run in bf16 (fp32/f32r moving operands stream at half rate
    on TRN2), accumulation and LayerNorm stay fp32
  - projection: q/k/v [n,512] tiles via PSUM accumulation over 8 K-tiles;
    per-head LN stats on DVE (bn_stats/bn_aggr), apply on ACT (Identity with
    per-partition scale/bias); PE-transposes of q,k into [d,n] layout run one
    n-tile behind the matmuls so the PE never waits on a fresh LN result
  - attention per head: scores^T[m,n] = k_h^T q_h; ACT computes exp(s/8)
    PSUM->SBUF in [128,1024] slabs (LN bounds |s| <= 8, so skipping the
    softmax max-subtraction is safe); v is augmented with a ones column so
    the PV matmul's row 64 accumulates the softmax denominators for free
  - the PV matmuls are software-pipelined two m-tiles behind the score
    matmuls: a PV matmul that waits on ACT's exp at the head of the PE FIFO
    keeps the PE throttled at 1.2 GHz (HAM never sees a gapless busy window);
    with the lag the PE streams continuously at 2.4 GHz and the phase is
    ACT-bound (the ~33M softmax exps are the hard floor)
  - normalize: numerators/denominators are drained out of PSUM early (frees
    the accumulator banks), one batched reciprocal per block on 32-aligned
    partition rows, and the partition-broadcast of 1/den goes through a DRAM
    bounce (SBUF APs cannot have zero partition step) - all off the PE queue
"""

import os
import sys

for _p in ("/opt/trn_rl_repo",):
    if _p not in sys.path:
        sys.path.insert(0, _p)

import ml_dtypes
import numpy as np

import concourse.bass as bass
import concourse.mybir as mybir
import concourse.tile as tile
from concourse.bass_utils import run_bass_kernel_spmd

F32 = mybir.dt.float32
F32R = mybir.dt.float32r
BF16 = mybir.dt.bfloat16

B = 4
NSEQ = 2048
DIM = 1024
NHEADS = 16
HDIM = 64
EPS = 1e-5

NCORES = 8
LAST_RESULTS = None
HG = 8            # heads per core
JW = HG * HDIM    # 512 output channels per core
KT = DIM // 128   # 8 contraction tiles for the projections


def split_multi_waits(nc, maxw=1):
    # TRN2 instructions carry a single sem-wait slot; this walrus build rejects
    # more. Tile's exit drain accumulates one wait per engine/DMA queue, so
    # hoist the excess onto injected NoOps just before the offending inst.
    for bb in nc.main_func.blocks:
        new_insts = []
        for inst in bb.instructions:
            si = inst.sync_info
            if si is not None and si.on_wait and len(si.on_wait) > maxw:
                waits = list(si.on_wait)
                extra, keep = waits[:-maxw], waits[-maxw:]
                for ci in range(0, len(extra), maxw):
                    nop = mybir.InstNoOp(
                        name=nc.get_next_instruction_name(), ins=[], outs=[],
                        sync_info=mybir.SyncInfo(
                            on_wait=extra[ci:ci + maxw], on_update=[]),
                    )
                    nop.engine = inst.engine
                    new_insts.append(nop)
                    nc.register_instruction(nop, overwrite=True)
                inst.sync_info = mybir.SyncInfo(
                    on_wait=keep, on_update=list(si.on_update))
            new_insts.append(inst)
        bb.instructions[:] = new_insts


def build(n_seq=NSEQ, has_bq=False, has_bkv=False, has_gbq=False, has_gbk=False):
    nt_n = n_seq // 128        # n tiles (16)
    sw = min(1024, n_seq)      # s-tile width (ACT exp granularity)
    nblk = n_seq // sw         # n blocks per head
    nch = sw // 512            # 512-wide output chunks per block
    scale = 1.0 / np.sqrt(HDIM)

    nc = bass.Bass()
    x1t = nc.dram_tensor("x1t", [DIM, n_seq], BF16, kind="ExternalInput")
    x2t = nc.dram_tensor("x2t", [DIM, n_seq], BF16, kind="ExternalInput")
    wq_d = nc.dram_tensor("wq", [DIM, JW], BF16, kind="ExternalInput")
    wk_d = nc.dram_tensor("wk", [DIM, JW], BF16, kind="ExternalInput")
    wv_d = nc.dram_tensor("wv", [DIM, JW], BF16, kind="ExternalInput")
    eye_d = nc.dram_tensor("eye", [128, 128], BF16, kind="ExternalInput")
    if has_bq:
        bq_d = nc.dram_tensor("bq", [JW], F32, kind="ExternalInput")
    if has_bkv:
        bk_d = nc.dram_tensor("bk", [JW], F32, kind="ExternalInput")
        bv_d = nc.dram_tensor("bv", [JW], F32, kind="ExternalInput")
    if has_gbq:
        gq_d = nc.dram_tensor("gq", [JW], F32, kind="ExternalInput")
        betq_d = nc.dram_tensor("betq", [JW], F32, kind="ExternalInput")
    if has_gbk:
        gk_d = nc.dram_tensor("gk", [JW], F32, kind="ExternalInput")
        betk_d = nc.dram_tensor("betk", [JW], F32, kind="ExternalInput")
    out_d = nc.dram_tensor("outT", [JW, n_seq], F32, kind="ExternalOutput")

    def bcast_from_dram(pool, vec_d, name):
        t = pool.tile([128, JW], F32, name=name)
        src = bass.AP(tensor=vec_d.tensor, offset=vec_d.offset,
                      ap=[[0, 128]] + list(vec_d.ap))
        nc.sync.dma_start(out=t, in_=src)
        return t

    with tile.TileContext(nc) as tc:
        with tc.tile_pool(name="persist", bufs=1) as persist:
            qT = persist.tile([128, 4, n_seq], BF16)   # [j, n] post-LN q
            kT = persist.tile([128, 4, n_seq], BF16)
            vA = persist.tile([128, nt_n, HG, HDIM + 1], BF16)  # v + ones col
            eye_sb = persist.tile([128, 128], BF16)
            eps_sb = persist.tile([128, 1], F32)
            nc.sync.dma_start(out=eye_sb, in_=eye_d[:, :])
            nc.vector.memset(eps_sb, EPS)
            nc.vector.memset(vA[:, :, :, HDIM:HDIM + 1], 1.0)

            bqb = bcast_from_dram(persist, bq_d[:], "bqb") if has_bq else None
            bkb = bcast_from_dram(persist, bk_d[:], "bkb") if has_bkv else None
            bvb = bcast_from_dram(persist, bv_d[:], "bvb") if has_bkv else None
            gqb = bcast_from_dram(persist, gq_d[:], "gqb") if has_gbq else None
            btqb = bcast_from_dram(persist, betq_d[:], "btqb") if has_gbq else None
            gkb = bcast_from_dram(persist, gk_d[:], "gkb") if has_gbk else None
            btkb = bcast_from_dram(persist, betk_d[:], "btkb") if has_gbk else None

            # ---------------- projection + LN + transpose ----------------
            with tc.tile_pool(name="wpool", bufs=1) as wpool, \
                 tc.tile_pool(name="lnb", bufs=3) as ln_pool, \
                 tc.tile_pool(name="stats", bufs=4) as st_pool, \
                 tc.tile_pool(name="pps", bufs=4, space="PSUM") as proj_ps, \
                 tc.tile_pool(name="tps", bufs=4, space="PSUM") as tp_ps:

                w_sb = {}
                for nm, dram in (("q", wq_d), ("k", wk_d), ("v", wv_d)):
                    w_sb[nm] = wpool.tile([128, KT, JW], BF16, name=f"w_{nm}")
                x1sb = wpool.tile([128, KT, n_seq], BF16, name="x1sb")
                x2sb = wpool.tile([128, KT, n_seq], BF16, name="x2sb")
                # DMA order matters: the first q matmul chain needs w_q and
                # the first x1 chunk only, so those go first
                nc.sync.dma_start(
                    out=w_sb["q"],
                    in_=wq_d.rearrange("(kt p) j -> p kt j", p=128))
                xq = n_seq // 4
                x1r = x1t.rearrange("(kt p) n -> p kt n", p=128)
                x2r = x2t.rearrange("(kt p) n -> p kt n", p=128)
                nc.sync.dma_start(out=x1sb[:, :, 0:xq], in_=x1r[:, :, 0:xq])
                nc.sync.dma_start(
                    out=w_sb["k"],
                    in_=wk_d.rearrange("(kt p) j -> p kt j", p=128))
                nc.sync.dma_start(out=x2sb[:, :, 0:xq], in_=x2r[:, :, 0:xq])
                nc.sync.dma_start(
                    out=w_sb["v"],
                    in_=wv_d.rearrange("(kt p) j -> p kt j", p=128))
                for xi in range(1, 4):
                    xs = slice(xi * xq, (xi + 1) * xq)
                    nc.sync.dma_start(out=x1sb[:, :, xs], in_=x1r[:, :, xs])
                    nc.sync.dma_start(out=x2sb[:, :, xs], in_=x2r[:, :, xs])

                def layernorm_into(psum, dst, bias_b, gb, bb_, use_act):
                    # per-head LN of a [128, 512] projection tile
                    if bias_b is not None:
                        src = ln_pool.tile([128, JW], F32, name="biased",
                                           tag="biased")
                        nc.vector.tensor_add(out=src, in0=psum, in1=bias_b)
                    else:
                        src = psum
                    stats = st_pool.tile([128, HG, 6], F32, name="stats")
                    for h in range(HG):
                        nc.vector.bn_stats(
                            out=stats[:, h, :],
                            in_=src[:, h * HDIM:(h + 1) * HDIM])
                    mv = st_pool.tile([128, HG, 2], F32, name="mv")
                    for h in range(HG):
                        nc.vector.bn_aggr(out=mv[:, h, :], in_=stats[:, h, :])
                    std = st_pool.tile([128, HG], F32, name="std")
                    nc.scalar.activation(
                        out=std, in_=mv[:, :, 1],
                        func=mybir.ActivationFunctionType.Sqrt,
                        bias=eps_sb, scale=1.0)
                    rstd = st_pool.tile([128, HG], F32, name="rstd")
                    nc.vector.reciprocal(out=rstd, in_=std)
                    # bias for the ACT apply: -mean * rstd
                    negmr = st_pool.tile([128, HG], F32, name="negmr")
                    nc.vector.tensor_mul(out=negmr, in0=mv[:, :, 0],
                                         in1=rstd)
                    nc.vector.tensor_scalar(
                        out=negmr, in0=negmr, scalar1=-1.0, scalar2=None,
                        op0=mybir.AluOpType.mult)
                    for h in range(HG):
                        if use_act:
                            # (q-mu)*rstd == q*rstd + (-mu*rstd), one ACT op
                            nc.scalar.activation(
                                out=dst[:, h * HDIM:(h + 1) * HDIM],
                                in_=src[:, h * HDIM:(h + 1) * HDIM],
                                func=mybir.ActivationFunctionType.Identity,
                                bias=negmr[:, h:h + 1], scale=rstd[:, h:h + 1])
                        else:
                            nc.vector.tensor_scalar(
                                out=dst[:, h * HDIM:(h + 1) * HDIM],
                                in0=src[:, h * HDIM:(h + 1) * HDIM],
                                scalar1=mv[:, h, 0:1],
                                scalar2=rstd[:, h:h + 1],
                                op0=mybir.AluOpType.subtract,
                                op1=mybir.AluOpType.mult)
                    if gb is not None:
                        nc.vector.tensor_mul(out=dst, in0=dst, in1=gb)
                        nc.vector.tensor_add(out=dst, in0=dst, in1=bb_)

                def emit_transposes(ln, dstT, nt):
                    nsl = slice(nt * 128, (nt + 1) * 128)
                    for jt in range(4):
                        tp = tp_ps.tile([128, 128], BF16, name="tp", tag="tp")
                        nc.tensor.transpose(
                            tp, ln[:, jt * 128:(jt + 1) * 128], eye_sb)
                        nc.any.tensor_copy(out=dstT[:, jt, nsl], in_=tp)

                # transposes run one n-tile behind the matmuls so the PE
                # never waits on a just-computed LN result
                pending = []
                for nt in range(nt_n):
                    nsl = slice(nt * 128, (nt + 1) * 128)
                    x1c = x1sb[:, :, nsl]
                    x2c = x2sb[:, :, nsl]

                    for nm, xc, dstT, bias_b, gb, bb_ in (
                        ("q", x1c, qT, bqb, gqb, btqb),
                        ("k", x2c, kT, bkb, gkb, btkb),
                    ):
                        ps = proj_ps.tile([128, JW], F32, name="ps", tag="ps")
                        for ct in range(KT):
                            nc.tensor.matmul(
                                ps, xc[:, ct, :], w_sb[nm][:, ct, :],
                                start=(ct == 0), stop=(ct == KT - 1))
                        ln = ln_pool.tile([128, JW], BF16, name="ln", tag="ln")
                        layernorm_into(ps, ln, bias_b, gb, bb_, True)
                        pending.append((ln, dstT, nt))

                    ps = proj_ps.tile([128, JW], F32, name="ps", tag="ps")
                    for ct in range(KT):
                        nc.tensor.matmul(
                            ps, x2c[:, ct, :], w_sb["v"][:, ct, :],
                            start=(ct == 0), stop=(ct == KT - 1))
                    psg = ps.rearrange("p (h d) -> p h d", h=HG)
                    if bvb is not None:
                        nc.vector.tensor_add(
                            out=vA[:, nt, :, 0:HDIM], in0=psg,
                            in1=bvb.rearrange("p (h d) -> p h d", h=HG))
                    else:
                        nc.vector.tensor_copy(out=vA[:, nt, :, 0:HDIM], in_=psg)
                    while len(pending) > 2:
                        emit_transposes(*pending.pop(0))
                for args in pending:
                    emit_transposes(*args)

            # ---------------- attention ----------------
            with tc.tile_pool(name="sps", bufs=2, space="PSUM") as s_ps, \
                 tc.tile_pool(name="pvps", bufs=2, space="PSUM") as pv_ps, \
                 tc.tile_pool(name="psb", bufs=3) as p_pool, \
                 tc.tile_pool(name="nrm", bufs=3) as n_pool, \
                 tc.tile_pool(name="dsc", bufs=4, space="DRAM") as dram_pool:
                # HAM re-warm: the projection tail idles the PE >3.4us
                # (final transposes wait on their LN chain), throttling it to
                # 1.2 GHz; the ACT-bound attention steady state never has a
                # gapless 3.4us PE window to un-throttle. Burn ~5us of dense
                # dummy matmuls so the whole attention phase runs at 2.4 GHz.
                warm = pv_ps.tile([128, 512], F32, name="warm", tag="pv")
                for wi in range(10):
                    nc.tensor.matmul(
                        warm, kT[0:64, 0, 0:128], qT[0:64, 0, 0:512],
                        start=True, stop=True)
                for h in range(HG):
                    pt, bp = divmod(h, 2)
                    prows = slice(bp * 64, (bp + 1) * 64)
                    kTh = kT[prows, pt, :]
                    qTh = qT[prows, pt, :]
                    for blk in range(nblk):
                        pvs = [pv_ps.tile([65, 512], F32, name=f"pv{c2}",
                                          tag="pv") for c2 in range(nch)]
                        # software pipeline: PV matmuls run two m-tiles behind
                        # the score matmuls, so the PE queue never stalls on
                        # ACT's exp (a stalled PE FIFO keeps HAM at 1.2 GHz)
                        LAG = 2
                        p_tiles = {}
                        for mi in range(nt_n + LAG):
                            if mi < nt_n:
                                s = s_ps.tile([128, sw], F32, name="s",
                                              tag="s", bufs=LAG + 1)
                                for c2 in range(nch):
                                    n0 = blk * sw + c2 * 512
                                    nc.tensor.matmul(
                                        s[:, c2 * 512:(c2 + 1) * 512],
                                        kTh[:, mi * 128:(mi + 1) * 128],
                                        qTh[:, n0:n0 + 512],
                                        start=True, stop=True)
                                p = p_pool.tile([128, sw], BF16, name="p",
                                                tag="p", bufs=LAG + 2)
                                nc.scalar.activation(
                                    out=p, in_=s,
                                    func=mybir.ActivationFunctionType.Exp,
                                    scale=float(scale))
                                p_tiles[mi] = p
                            if mi >= LAG:
                                m = mi - LAG
                                p = p_tiles.pop(m)
                                for c2 in range(nch):
                                    nc.tensor.matmul(
                                        pvs[c2], vA[:, m, h, :],
                                        p[:, c2 * 512:(c2 + 1) * 512],
                                        start=(m == 0), stop=(m == nt_n - 1))
                        # Drain numerators + denominator rows out of PSUM
                        # first (frees the pv banks for the next block), then
                        # one batched reciprocal per block: rows parked at
                        # partitions 0/32 (compute APs need 32-aligned bases).
                        # The partition-broadcast goes through a DRAM bounce
                        # (SBUF APs cannot have a zero partition step) so the
                        # PE never has to wait on this chain.
                        dens = n_pool.tile([64, 512], F32, name="dens",
                                           tag="dens")
                        nc.vector.memset(dens, 1.0)
                        osb0s = []
                        for c2 in range(nch):
                            osb0 = n_pool.tile([64, 512], F32, name="osb0",
                                               tag="osb0", bufs=4)
                            nc.vector.tensor_copy(out=osb0, in_=pvs[c2][0:64, :])
                            nc.scalar.copy(out=dens[32 * c2:32 * c2 + 1, :],
                                           in_=pvs[c2][64:65, :])
                            osb0s.append(osb0)
                        denr = n_pool.tile([64, 512], F32, name="denr",
                                           tag="denr")
                        nc.vector.reciprocal(out=denr, in_=dens)
                        for c2 in range(nch):
                            dscr = dram_pool.tile([512], F32, name="dscr",
                                                  tag="dscr")
                            nc.sync.dma_start(
                                out=dscr, in_=denr[32 * c2:32 * c2 + 1, :])
                            denb_sb = n_pool.tile([64, 512], F32,
                                                  name="denb_sb",
                                                  tag="denb_sb")
                            dscr_b = bass.AP(tensor=dscr.tensor,
                                             offset=dscr.offset,
                                             ap=[[0, 64]] + list(dscr.ap))
                            nc.sync.dma_start(out=denb_sb, in_=dscr_b)
                            osb = n_pool.tile([64, 512], F32, name="osb",
                                              tag="osb")
                            nc.vector.tensor_mul(
                                out=osb, in0=osb0s[c2], in1=denb_sb)
                            n0 = blk * sw + c2 * 512
                            nc.sync.dma_start(
                                out=out_d[h * HDIM:(h + 1) * HDIM,
                                          n0:n0 + 512],
                                in_=osb)
    split_multi_waits(nc)
    return nc


def shard_inputs(x1, x2, wq, bq, wkv, bkv, gamma_q, beta_q, gamma_k, beta_k,
                 flags, n_seq=NSEQ):
    has_bq, has_bkv, has_gbq, has_gbk = flags
    bf16 = ml_dtypes.bfloat16
    eye = np.eye(128, dtype=bf16)
    in_maps = []
    for core in range(NCORES):
        b, g = divmod(core, 2)
        jsl = slice(g * JW, (g + 1) * JW)
        m = {
            "x1t": np.ascontiguousarray(x1[b, :n_seq].T.astype(bf16)),
            "x2t": np.ascontiguousarray(x2[b, :n_seq].T.astype(bf16)),
            "wq": np.ascontiguousarray(wq[:, jsl].astype(bf16)),
            "wk": np.ascontiguousarray(wkv[:, jsl].astype(bf16)),
            "wv": np.ascontiguousarray(
                wkv[:, DIM + g * JW:DIM + (g + 1) * JW].astype(bf16)),
            "eye": eye,
        }
        if has_bq:
            m["bq"] = np.ascontiguousarray(bq[jsl])
        if has_bkv:
            m["bk"] = np.ascontiguousarray(bkv[jsl])
            m["bv"] = np.ascontiguousarray(bkv[DIM + g * JW:DIM + (g + 1) * JW])
        if has_gbq:
            m["gq"] = np.tile(gamma_q, HG).astype(np.float32)
            m["betq"] = np.tile(beta_q, HG).astype(np.float32)
        if has_gbk:
            m["gk"] = np.tile(gamma_k, HG).astype(np.float32)
            m["betk"] = np.tile(beta_k, HG).astype(np.float32)
        in_maps.append(m)
    return in_maps


def kernel(x1, x2, wq, bq, wkv, bkv, gamma_q, beta_q, gamma_k, beta_k):
    x1 = np.asarray(x1, dtype=np.float32)
    x2 = np.asarray(x2, dtype=np.float32)
    wq = np.asarray(wq, dtype=np.float32)
    bq = np.asarray(bq, dtype=np.float32)
    wkv = np.asarray(wkv, dtype=np.float32)
    bkv = np.asarray(bkv, dtype=np.float32)
    gamma_q = np.asarray(gamma_q, dtype=np.float32)
    beta_q = np.asarray(beta_q, dtype=np.float32)
    gamma_k = np.asarray(gamma_k, dtype=np.float32)
    beta_k = np.asarray(beta_k, dtype=np.float32)

    flags = (
        bool(np.any(bq)),
        bool(np.any(bkv)),
        not (np.all(gamma_q == 1.0) and np.all(beta_q == 0.0)),
        not (np.all(gamma_k == 1.0) and np.all(beta_k == 0.0)),
    )
    nc = build(NSEQ, *flags)
    in_maps = shard_inputs(x1, x2, wq, bq, wkv, bkv, gamma_q, beta_q,
                           gamma_k, beta_k, flags)
    trace = bool(int(os.environ.get("KERNEL_TRACE", "0")))
    res = run_bass_kernel_spmd(nc, in_maps, core_ids=list(range(NCORES)),
                               trace=trace)
    global LAST_RESULTS
    LAST_RESULTS = res
    out = np.empty((B, NSEQ, DIM), dtype=np.float32)
    for core in range(NCORES):
        b, g = divmod(core, 2)
        out[b, :, g * JW:(g + 1) * JW] = res.results[core]["outT"].T
    return out


# revision 33
# speedup vs baseline: 1.0541x; 1.0541x over previous
"""Fused AttnBlock kernel for 8 Trainium2 NeuronCores.

Problem: q = LN_head(x1 @ wq + bq), k = LN_head(x2 @ wk + bk), v = x2 @ wv + bv,
out = softmax(q k^T / sqrt(D)) v, with B=4, N=2048, C=1024, H=16, D=64.

Sharding: data-parallel over batch (4) x tensor-parallel over head groups (2).
Each core handles one (batch, head-group) pair fully locally: its 8 heads'
columns of wq/wkv are contiguous slices, so there are no collectives; the host
scatters inputs (pre-transposed + cast to bf16) and gathers/transposes outputs.

Per-core dataflow:
  - host pre-transposes x1/x2 so the contraction dim (C) lands on partitions;
    all big matmuls run in bf16 (fp32/f32r moving operands stream at half rate
    on TRN2), accumulation and LayerNorm stay fp32
  - projection: q/k/v [n,512] tiles via PSUM accumulation over 8 K-tiles;
    per-head LN stats on DVE (bn_stats/bn_aggr), apply on ACT (Identity with
    per-partition scale/bias); PE-transposes of q,k into [d,n] layout run one
    n-tile behind the matmuls so the PE never waits on a fresh LN result
  - attention per head: scores^T[m,n] = k_h^T q_h; ACT computes exp(s/8)
    PSUM->SBUF in [128,1024] slabs (LN bounds |s| <= 8, so skipping the
    softmax max-subtraction is safe); v is augmented with a ones column so
    the PV matmul's row 64 accumulates the softmax denominators for free
  - the PV matmuls are software-pipelined two m-tiles behind the score
    matmuls: a PV matmul that waits on ACT's exp at the head of the PE FIFO
    keeps the PE throttled at 1.2 GHz (HAM never sees a gapless busy window);
    with the lag the PE streams continuously at 2.4 GHz and the phase is
    ACT-bound (the ~33M softmax exps are the hard floor)
  - normalize: numerators/denominators are drained out of PSUM early (frees
    the accumulator banks), one batched reciprocal per block on 32-aligned
    partition rows, and the partition-broadcast of 1/den goes through a DRAM
    bounce (SBUF APs cannot have zero partition step) - all off the PE queue
"""

import os
import sys

for _p in ("/opt/trn_rl_repo",):
    if _p not in sys.path:
        sys.path.insert(0, _p)

import ml_dtypes
import numpy as np

import concourse.bass as bass
import concourse.mybir as mybir
import concourse.tile as tile
from concourse.bass_utils import run_bass_kernel_spmd

F32 = mybir.dt.float32
F32R = mybir.dt.float32r
BF16 = mybir.dt.bfloat16

B = 4
NSEQ = 2048
DIM = 1024
NHEADS = 16
HDIM = 64
EPS = 1e-5

NCORES = 8
LAST_RESULTS = None
HG = 8            # heads per core
JW = HG * HDIM    # 512 output channels per core
KT = DIM // 128   # 8 contraction tiles for the projections


def split_multi_waits(nc, maxw=1):
    # TRN2 instructions carry a single sem-wait slot; this walrus build rejects
    # more. Tile's exit drain accumulates one wait per engine/DMA queue, so
    # hoist the excess onto injected NoOps just before the offending inst.
    for bb in nc.main_func.blocks:
        new_insts = []
        for inst in bb.instructions:
            si = inst.sync_info
            if si is not None and si.on_wait and len(si.on_wait) > maxw:
                waits = list(si.on_wait)
                extra, keep = waits[:-maxw], waits[-maxw:]
                for ci in range(0, len(extra), maxw):
                    nop = mybir.InstNoOp(
                        name=nc.get_next_instruction_name(), ins=[], outs=[],
                        sync_info=mybir.SyncInfo(
                            on_wait=extra[ci:ci + maxw], on_update=[]),
                    )
                    nop.engine = inst.engine
                    new_insts.append(nop)
                    nc.register_instruction(nop, overwrite=True)
                inst.sync_info = mybir.SyncInfo(
                    on_wait=keep, on_update=list(si.on_update))
            new_insts.append(inst)
        bb.instructions[:] = new_insts


def build(n_seq=NSEQ, has_bq=False, has_bkv=False, has_gbq=False, has_gbk=False):
    nt_n = n_seq // 128        # n tiles (16)
    sw = min(1024, n_seq)      # s-tile width (ACT exp granularity)
    nblk = n_seq // sw         # n blocks per head
    nch = sw // 512            # 512-wide output chunks per block
    scale = 1.0 / np.sqrt(HDIM)

    nc = bass.Bass()
    x1t = nc.dram_tensor("x1t", [DIM, n_seq], BF16, kind="ExternalInput")
    x2t = nc.dram_tensor("x2t", [DIM, n_seq], BF16, kind="ExternalInput")
    wq_d = nc.dram_tensor("wq", [DIM, JW], BF16, kind="ExternalInput")
    wk_d = nc.dram_tensor("wk", [DIM, JW], BF16, kind="ExternalInput")
    wv_d = nc.dram_tensor("wv", [DIM, JW], BF16, kind="ExternalInput")
    eye_d = nc.dram_tensor("eye", [128, 128], BF16, kind="ExternalInput")
    if has_bq:
        bq_d = nc.dram_tensor("bq", [JW], F32, kind="ExternalInput")
    if has_bkv:
        bk_d = nc.dram_tensor("bk", [JW], F32, kind="ExternalInput")
        bv_d = nc.dram_tensor("bv", [JW], F32, kind="ExternalInput")
    if has_gbq:
        gq_d = nc.dram_tensor("gq", [JW], F32, kind="ExternalInput")
        betq_d = nc.dram_tensor("betq", [JW], F32, kind="ExternalInput")
    if has_gbk:
        gk_d = nc.dram_tensor("gk", [JW], F32, kind="ExternalInput")
        betk_d = nc.dram_tensor("betk", [JW], F32, kind="ExternalInput")
    out_d = nc.dram_tensor("outT", [JW, n_seq], F32, kind="ExternalOutput")

    def bcast_from_dram(pool, vec_d, name):
        t = pool.tile([128, JW], F32, name=name)
        src = bass.AP(tensor=vec_d.tensor, offset=vec_d.offset,
                      ap=[[0, 128]] + list(vec_d.ap))
        nc.sync.dma_start(out=t, in_=src)
        return t

    with tile.TileContext(nc) as tc:
        with tc.tile_pool(name="persist", bufs=1) as persist:
            qT = persist.tile([128, 4, n_seq], BF16)   # [j, n] post-LN q
            kT = persist.tile([128, 4, n_seq], BF16)
            vA = persist.tile([128, nt_n, HG, HDIM + 1], BF16)  # v + ones col
            eye_sb = persist.tile([128, 128], BF16)
            eps_sb = persist.tile([128, 1], F32)
            nc.sync.dma_start(out=eye_sb, in_=eye_d[:, :])
            nc.vector.memset(eps_sb, EPS)
            nc.vector.memset(vA[:, :, :, HDIM:HDIM + 1], 1.0)

            bqb = bcast_from_dram(persist, bq_d[:], "bqb") if has_bq else None
            bkb = bcast_from_dram(persist, bk_d[:], "bkb") if has_bkv else None
            bvb = bcast_from_dram(persist, bv_d[:], "bvb") if has_bkv else None
            gqb = bcast_from_dram(persist, gq_d[:], "gqb") if has_gbq else None
            btqb = bcast_from_dram(persist, betq_d[:], "btqb") if has_gbq else None
            gkb = bcast_from_dram(persist, gk_d[:], "gkb") if has_gbk else None
            btkb = bcast_from_dram(persist, betk_d[:], "btkb") if has_gbk else None

            # ---------------- projection + LN + transpose ----------------
            with tc.tile_pool(name="wpool", bufs=1) as wpool, \
                 tc.tile_pool(name="lnb", bufs=3) as ln_pool, \
                 tc.tile_pool(name="stats", bufs=4) as st_pool, \
                 tc.tile_pool(name="pps", bufs=6, space="PSUM") as proj_ps, \
                 tc.tile_pool(name="tps", bufs=2, space="PSUM") as tp_ps:

                w_sb = {}
                for nm, dram in (("q", wq_d), ("k", wk_d), ("v", wv_d)):
                    w_sb[nm] = wpool.tile([128, KT, JW], BF16, name=f"w_{nm}")
                x1sb = wpool.tile([128, KT, n_seq], BF16, name="x1sb")
                x2sb = wpool.tile([128, KT, n_seq], BF16, name="x2sb")
                # DMA order matters: the first q matmul chain needs w_q and
                # the first x1 chunk only, so those go first
                nc.sync.dma_start(
                    out=w_sb["q"],
                    in_=wq_d.rearrange("(kt p) j -> p kt j", p=128))
                xq = n_seq // 4
                x1r = x1t.rearrange("(kt p) n -> p kt n", p=128)
                x2r = x2t.rearrange("(kt p) n -> p kt n", p=128)
                nc.sync.dma_start(out=x1sb[:, :, 0:xq], in_=x1r[:, :, 0:xq])
                nc.sync.dma_start(
                    out=w_sb["k"],
                    in_=wk_d.rearrange("(kt p) j -> p kt j", p=128))
                nc.sync.dma_start(out=x2sb[:, :, 0:xq], in_=x2r[:, :, 0:xq])
                nc.sync.dma_start(
                    out=w_sb["v"],
                    in_=wv_d.rearrange("(kt p) j -> p kt j", p=128))
                for xi in range(1, 4):
                    xs = slice(xi * xq, (xi + 1) * xq)
                    nc.sync.dma_start(out=x1sb[:, :, xs], in_=x1r[:, :, xs])
                    nc.sync.dma_start(out=x2sb[:, :, xs], in_=x2r[:, :, xs])

                def layernorm_into(psum, dst, bias_b, gb, bb_, use_act):
                    # per-head LN of a [128, 512] projection tile
                    if bias_b is not None:
                        src = ln_pool.tile([128, JW], F32, name="biased",
                                           tag="biased")
                        nc.vector.tensor_add(out=src, in0=psum, in1=bias_b)
                    else:
                        src = psum
                    stats = st_pool.tile([128, HG, 6], F32, name="stats")
                    for h in range(HG):
                        nc.vector.bn_stats(
                            out=stats[:, h, :],
                            in_=src[:, h * HDIM:(h + 1) * HDIM])
                    mv = st_pool.tile([128, HG, 2], F32, name="mv")
                    for h in range(HG):
                        nc.vector.bn_aggr(out=mv[:, h, :], in_=stats[:, h, :])
                    std = st_pool.tile([128, HG], F32, name="std")
                    nc.scalar.activation(
                        out=std, in_=mv[:, :, 1],
                        func=mybir.ActivationFunctionType.Sqrt,
                        bias=eps_sb, scale=1.0)
                    rstd = st_pool.tile([128, HG], F32, name="rstd")
                    nc.vector.reciprocal(out=rstd, in_=std)
                    # bias for the ACT apply: -mean * rstd
                    negmr = st_pool.tile([128, HG], F32, name="negmr")
                    nc.vector.tensor_mul(out=negmr, in0=mv[:, :, 0],
                                         in1=rstd)
                    nc.vector.tensor_scalar(
                        out=negmr, in0=negmr, scalar1=-1.0, scalar2=None,
                        op0=mybir.AluOpType.mult)
                    for h in range(HG):
                        if use_act:
                            # (q-mu)*rstd == q*rstd + (-mu*rstd), one ACT op
                            nc.scalar.activation(
                                out=dst[:, h * HDIM:(h + 1) * HDIM],
                                in_=src[:, h * HDIM:(h + 1) * HDIM],
                                func=mybir.ActivationFunctionType.Identity,
                                bias=negmr[:, h:h + 1], scale=rstd[:, h:h + 1])
                        else:
                            nc.vector.tensor_scalar(
                                out=dst[:, h * HDIM:(h + 1) * HDIM],
                                in0=src[:, h * HDIM:(h + 1) * HDIM],
                                scalar1=mv[:, h, 0:1],
                                scalar2=rstd[:, h:h + 1],
                                op0=mybir.AluOpType.subtract,
                                op1=mybir.AluOpType.mult)
                    if gb is not None:
                        nc.vector.tensor_mul(out=dst, in0=dst, in1=gb)
                        nc.vector.tensor_add(out=dst, in0=dst, in1=bb_)

                def emit_transposes(ln, dstT, nt):
                    nsl = slice(nt * 128, (nt + 1) * 128)
                    for jt in range(4):
                        tp = tp_ps.tile([128, 128], BF16, name="tp", tag="tp")
                        nc.tensor.transpose(
                            tp, ln[:, jt * 128:(jt + 1) * 128], eye_sb)
                        nc.any.tensor_copy(out=dstT[:, jt, nsl], in_=tp)

                # transposes run one n-tile behind the matmuls so the PE
                # never waits on a just-computed LN result
                pending = []
                for nt in range(nt_n):
                    nsl = slice(nt * 128, (nt + 1) * 128)
                    x1c = x1sb[:, :, nsl]
                    x2c = x2sb[:, :, nsl]

                    for nm, xc, dstT, bias_b, gb, bb_ in (
                        ("q", x1c, qT, bqb, gqb, btqb),
                        ("k", x2c, kT, bkb, gkb, btkb),
                    ):
                        ps = proj_ps.tile([128, JW], F32, name="ps", tag="ps")
                        for ct in range(KT):
                            nc.tensor.matmul(
                                ps, xc[:, ct, :], w_sb[nm][:, ct, :],
                                start=(ct == 0), stop=(ct == KT - 1))
                        ln = ln_pool.tile([128, JW], BF16, name="ln", tag="ln")
                        layernorm_into(ps, ln, bias_b, gb, bb_, True)
                        pending.append((ln, dstT, nt))

                    ps = proj_ps.tile([128, JW], F32, name="ps", tag="ps")
                    for ct in range(KT):
                        nc.tensor.matmul(
                            ps, x2c[:, ct, :], w_sb["v"][:, ct, :],
                            start=(ct == 0), stop=(ct == KT - 1))
                    psg = ps.rearrange("p (h d) -> p h d", h=HG)
                    if bvb is not None:
                        nc.vector.tensor_add(
                            out=vA[:, nt, :, 0:HDIM], in0=psg,
                            in1=bvb.rearrange("p (h d) -> p h d", h=HG))
                    else:
                        nc.vector.tensor_copy(out=vA[:, nt, :, 0:HDIM], in_=psg)
                    while len(pending) > 2:
                        emit_transposes(*pending.pop(0))
                for args in pending:
                    emit_transposes(*args)

            # ---------------- attention ----------------
            with tc.tile_pool(name="sps", bufs=2, space="PSUM") as s_ps, \
                 tc.tile_pool(name="pvps", bufs=2, space="PSUM") as pv_ps, \
                 tc.tile_pool(name="psb", bufs=3) as p_pool, \
                 tc.tile_pool(name="nrm", bufs=3) as n_pool, \
                 tc.tile_pool(name="dsc", bufs=4, space="DRAM") as dram_pool:
                for h in range(HG):
                    pt, bp = divmod(h, 2)
                    prows = slice(bp * 64, (bp + 1) * 64)
                    kTh = kT[prows, pt, :]
                    qTh = qT[prows, pt, :]
                    for blk in range(nblk):
                        pvs = [pv_ps.tile([65, 512], F32, name=f"pv{c2}",
                                          tag="pv") for c2 in range(nch)]
                        # software pipeline: PV matmuls run two m-tiles behind
                        # the score matmuls, so the PE queue never stalls on
                        # ACT's exp (a stalled PE FIFO keeps HAM at 1.2 GHz)
                        LAG = 2
                        p_tiles = {}
                        for mi in range(nt_n + LAG):
                            if mi < nt_n:
                                s = s_ps.tile([128, sw], F32, name="s",
                                              tag="s", bufs=LAG + 1)
                                for c2 in range(nch):
                                    n0 = blk * sw + c2 * 512
                                    nc.tensor.matmul(
                                        s[:, c2 * 512:(c2 + 1) * 512],
                                        kTh[:, mi * 128:(mi + 1) * 128],
                                        qTh[:, n0:n0 + 512],
                                        start=True, stop=True)
                                p = p_pool.tile([128, sw], BF16, name="p",
                                                tag="p", bufs=LAG + 2)
                                nc.scalar.activation(
                                    out=p, in_=s,
                                    func=mybir.ActivationFunctionType.Exp,
                                    scale=float(scale))
                                p_tiles[mi] = p
                            if mi >= LAG:
                                m = mi - LAG
                                p = p_tiles.pop(m)
                                for c2 in range(nch):
                                    nc.tensor.matmul(
                                        pvs[c2], vA[:, m, h, :],
                                        p[:, c2 * 512:(c2 + 1) * 512],
                                        start=(m == 0), stop=(m == nt_n - 1))
                        # Drain numerators + denominator rows out of PSUM
                        # first (frees the pv banks for the next block), then
                        # one batched reciprocal per block: rows parked at
                        # partitions 0/32 (compute APs need 32-aligned bases).
                        # The partition-broadcast goes through a DRAM bounce
                        # (SBUF APs cannot have a zero partition step) so the
                        # PE never has to wait on this chain.
                        dens = n_pool.tile([64, 512], F32, name="dens",
                                           tag="dens")
                        nc.vector.memset(dens, 1.0)
                        osb0s = []
                        for c2 in range(nch):
                            osb0 = n_pool.tile([64, 512], F32, name="osb0",
                                               tag="osb0", bufs=4)
                            nc.vector.tensor_copy(out=osb0, in_=pvs[c2][0:64, :])
                            nc.scalar.copy(out=dens[32 * c2:32 * c2 + 1, :],
                                           in_=pvs[c2][64:65, :])
                            osb0s.append(osb0)
                        denr = n_pool.tile([64, 512], F32, name="denr",
                                           tag="denr")
                        nc.vector.reciprocal(out=denr, in_=dens)
                        for c2 in range(nch):
                            dscr = dram_pool.tile([512], F32, name="dscr",
                                                  tag="dscr")
                            nc.sync.dma_start(
                                out=dscr, in_=denr[32 * c2:32 * c2 + 1, :])
                            denb_sb = n_pool.tile([64, 512], F32,
                                                  name="denb_sb",
                                                  tag="denb_sb")
                            dscr_b = bass.AP(tensor=dscr.tensor,
                                             offset=dscr.offset,
                                             ap=[[0, 64]] + list(dscr.ap))
                            nc.sync.dma_start(out=denb_sb, in_=dscr_b)
                            osb = n_pool.tile([64, 512], F32, name="osb",
                                              tag="osb")
                            nc.vector.tensor_mul(
                                out=osb, in0=osb0s[c2], in1=denb_sb)
                            n0 = blk * sw + c2 * 512
                            nc.sync.dma_start(
                                out=out_d[h * HDIM:(h + 1) * HDIM,
                                          n0:n0 + 512],
                                in_=osb)
    split_multi_waits(nc)
    return nc


def shard_inputs(x1, x2, wq, bq, wkv, bkv, gamma_q, beta_q, gamma_k, beta_k,
                 flags, n_seq=NSEQ):
    has_bq, has_bkv, has_gbq, has_gbk = flags
    bf16 = ml_dtypes.bfloat16
    eye = np.eye(128, dtype=bf16)
    in_maps = []
    for core in range(NCORES):
        b, g = divmod(core, 2)
        jsl = slice(g * JW, (g + 1) * JW)
        m = {
            "x1t": np.ascontiguousarray(x1[b, :n_seq].T.astype(bf16)),
            "x2t": np.ascontiguousarray(x2[b, :n_seq].T.astype(bf16)),
            "wq": np.ascontiguousarray(wq[:, jsl].astype(bf16)),
            "wk": np.ascontiguousarray(wkv[:, jsl].astype(bf16)),
            "wv": np.ascontiguousarray(
                wkv[:, DIM + g * JW:DIM + (g + 1) * JW].astype(bf16)),
            "eye": eye,
        }
        if has_bq:
            m["bq"] = np.ascontiguousarray(bq[jsl])
        if has_bkv:
            m["bk"] = np.ascontiguousarray(bkv[jsl])
            m["bv"] = np.ascontiguousarray(bkv[DIM + g * JW:DIM + (g + 1) * JW])
        if has_gbq:
            m["gq"] = np.tile(gamma_q, HG).astype(np.float32)
            m["betq"] = np.tile(beta_q, HG).astype(np.float32)
        if has_gbk:
            m["gk"] = np.tile(gamma_k, HG).astype(np.float32)
            m["betk"] = np.tile(beta_k, HG).astype(np.float32)
        in_maps.append(m)
    return in_maps


def kernel(x1, x2, wq, bq, wkv, bkv, gamma_q, beta_q, gamma_k, beta_k):
    x1 = np.asarray(x1, dtype=np.float32)
    x2 = np.asarray(x2, dtype=np.float32)
    wq = np.asarray(wq, dtype=np.float32)
    bq = np.asarray(bq, dtype=np.float32)
    wkv = np.asarray(wkv, dtype=np.float32)
    bkv = np.asarray(bkv, dtype=np.float32)
    gamma_q = np.asarray(gamma_q, dtype=np.float32)
    beta_q = np.asarray(beta_q, dtype=np.float32)
    gamma_k = np.asarray(gamma_k, dtype=np.float32)
    beta_k = np.asarray(beta_k, dtype=np.float32)

    flags = (
        bool(np.any(bq)),
        bool(np.any(bkv)),
        not (np.all(gamma_q == 1.0) and np.all(beta_q == 0.0)),
        not (np.all(gamma_k == 1.0) and np.all(beta_k == 0.0)),
    )
    nc = build(NSEQ, *flags)
    in_maps = shard_inputs(x1, x2, wq, bq, wkv, bkv, gamma_q, beta_q,
                           gamma_k, beta_k, flags)
    trace = bool(int(os.environ.get("KERNEL_TRACE", "0")))
    res = run_bass_kernel_spmd(nc, in_maps, core_ids=list(range(NCORES)),
                               trace=trace)
    global LAST_RESULTS
    LAST_RESULTS = res
    out = np.empty((B, NSEQ, DIM), dtype=np.float32)
    for core in range(NCORES):
        b, g = divmod(core, 2)
        out[b, :, g * JW:(g + 1) * JW] = res.results[core]["outT"].T
    return out


# revision 34
# speedup vs baseline: 1.0902x; 1.0342x over previous
"""Fused AttnBlock kernel for 8 Trainium2 NeuronCores.

Problem: q = LN_head(x1 @ wq + bq), k = LN_head(x2 @ wk + bk), v = x2 @ wv + bv,
out = softmax(q k^T / sqrt(D)) v, with B=4, N=2048, C=1024, H=16, D=64.

Sharding: data-parallel over batch (4) x tensor-parallel over head groups (2).
Each core handles one (batch, head-group) pair fully locally: its 8 heads'
columns of wq/wkv are contiguous slices, so there are no collectives; the host
scatters inputs (pre-transposed + cast to bf16) and gathers/transposes outputs.

Per-core dataflow:
  - host pre-transposes x1/x2 so the contraction dim (C) lands on partitions;
    all big matmuls run in bf16 (fp32/f32r moving operands stream at half rate
    on TRN2), accumulation and LayerNorm stay fp32
  - projection: q/k/v [n,512] tiles via PSUM accumulation over 8 K-tiles;
    per-head LN stats on DVE (bn_stats/bn_aggr), apply on ACT (Identity with
    per-partition scale/bias); PE-transposes of q,k into [d,n] layout run one
    n-tile behind the matmuls so the PE never waits on a fresh LN result
  - attention per head: scores^T[m,n] = k_h^T q_h; ACT computes exp(s/8)
    PSUM->SBUF in [128,1024] slabs (LN bounds |s| <= 8, so skipping the
    softmax max-subtraction is safe); v is augmented with a ones column so
    the PV matmul's row 64 accumulates the softmax denominators for free
  - the PV matmuls are software-pipelined two m-tiles behind the score
    matmuls: a PV matmul that waits on ACT's exp at the head of the PE FIFO
    keeps the PE throttled at 1.2 GHz (HAM never sees a gapless busy window);
    with the lag the PE streams continuously at 2.4 GHz and the phase is
    ACT-bound (the ~33M softmax exps are the hard floor)
  - normalize: numerators/denominators are drained out of PSUM early (frees
    the accumulator banks), one batched reciprocal per block on 32-aligned
    partition rows, and the partition-broadcast of 1/den goes through a DRAM
    bounce (SBUF APs cannot have zero partition step) - all off the PE queue
"""

import os
import sys

for _p in ("/opt/trn_rl_repo",):
    if _p not in sys.path:
        sys.path.insert(0, _p)

import ml_dtypes
import numpy as np

import concourse.bass as bass
import concourse.mybir as mybir
import concourse.tile as tile
from concourse.bass_utils import run_bass_kernel_spmd

F32 = mybir.dt.float32
F32R = mybir.dt.float32r
BF16 = mybir.dt.bfloat16

B = 4
NSEQ = 2048
DIM = 1024
NHEADS = 16
HDIM = 64
EPS = 1e-5

NCORES = 8
LAST_RESULTS = None
HG = 8            # heads per core
JW = HG * HDIM    # 512 output channels per core
KT = DIM // 128   # 8 contraction tiles for the projections


def split_multi_waits(nc, maxw=1):
    # TRN2 instructions carry a single sem-wait slot; this walrus build rejects
    # more. Tile's exit drain accumulates one wait per engine/DMA queue, so
    # hoist the excess onto injected NoOps just before the offending inst.
    for bb in nc.main_func.blocks:
        new_insts = []
        for inst in bb.instructions:
            si = inst.sync_info
            if si is not None and si.on_wait and len(si.on_wait) > maxw:
                waits = list(si.on_wait)
                extra, keep = waits[:-maxw], waits[-maxw:]
                for ci in range(0, len(extra), maxw):
                    nop = mybir.InstNoOp(
                        name=nc.get_next_instruction_name(), ins=[], outs=[],
                        sync_info=mybir.SyncInfo(
                            on_wait=extra[ci:ci + maxw], on_update=[]),
                    )
                    nop.engine = inst.engine
                    new_insts.append(nop)
                    nc.register_instruction(nop, overwrite=True)
                inst.sync_info = mybir.SyncInfo(
                    on_wait=keep, on_update=list(si.on_update))
            new_insts.append(inst)
        bb.instructions[:] = new_insts


def build(n_seq=NSEQ, has_bq=False, has_bkv=False, has_gbq=False, has_gbk=False):
    nt_n = n_seq // 128        # n tiles (16)
    sw = min(1024, n_seq)      # s-tile width (ACT exp granularity)
    nblk = n_seq // sw         # n blocks per head
    nch = sw // 512            # 512-wide output chunks per block
    scale = 1.0 / np.sqrt(HDIM)

    nc = bass.Bass()
    x1t = nc.dram_tensor("x1t", [DIM, n_seq], BF16, kind="ExternalInput")
    x2t = nc.dram_tensor("x2t", [DIM, n_seq], BF16, kind="ExternalInput")
    wq_d = nc.dram_tensor("wq", [DIM, JW], BF16, kind="ExternalInput")
    wk_d = nc.dram_tensor("wk", [DIM, JW], BF16, kind="ExternalInput")
    wv_d = nc.dram_tensor("wv", [DIM, JW], BF16, kind="ExternalInput")
    eye_d = nc.dram_tensor("eye", [128, 128], BF16, kind="ExternalInput")
    if has_bq:
        bq_d = nc.dram_tensor("bq", [JW], F32, kind="ExternalInput")
    if has_bkv:
        bk_d = nc.dram_tensor("bk", [JW], F32, kind="ExternalInput")
        bv_d = nc.dram_tensor("bv", [JW], F32, kind="ExternalInput")
    if has_gbq:
        gq_d = nc.dram_tensor("gq", [JW], F32, kind="ExternalInput")
        betq_d = nc.dram_tensor("betq", [JW], F32, kind="ExternalInput")
    if has_gbk:
        gk_d = nc.dram_tensor("gk", [JW], F32, kind="ExternalInput")
        betk_d = nc.dram_tensor("betk", [JW], F32, kind="ExternalInput")
    out_d = nc.dram_tensor("outT", [JW, n_seq], F32, kind="ExternalOutput")

    def bcast_from_dram(pool, vec_d, name):
        t = pool.tile([128, JW], F32, name=name)
        src = bass.AP(tensor=vec_d.tensor, offset=vec_d.offset,
                      ap=[[0, 128]] + list(vec_d.ap))
        nc.sync.dma_start(out=t, in_=src)
        return t

    with tile.TileContext(nc) as tc:
        with tc.tile_pool(name="persist", bufs=1) as persist:
            qT = persist.tile([128, 4, n_seq], BF16)   # [j, n] post-LN q
            kT = persist.tile([128, 4, n_seq], BF16)
            vA = persist.tile([128, nt_n, HG, HDIM + 1], BF16)  # v + ones col
            eye_sb = persist.tile([128, 128], BF16)
            eps_sb = persist.tile([128, 1], F32)
            nc.sync.dma_start(out=eye_sb, in_=eye_d[:, :])
            nc.vector.memset(eps_sb, EPS)
            nc.vector.memset(vA[:, :, :, HDIM:HDIM + 1], 1.0)

            bqb = bcast_from_dram(persist, bq_d[:], "bqb") if has_bq else None
            bkb = bcast_from_dram(persist, bk_d[:], "bkb") if has_bkv else None
            bvb = bcast_from_dram(persist, bv_d[:], "bvb") if has_bkv else None
            gqb = bcast_from_dram(persist, gq_d[:], "gqb") if has_gbq else None
            btqb = bcast_from_dram(persist, betq_d[:], "btqb") if has_gbq else None
            gkb = bcast_from_dram(persist, gk_d[:], "gkb") if has_gbk else None
            btkb = bcast_from_dram(persist, betk_d[:], "btkb") if has_gbk else None

            # ---------------- projection + LN + transpose ----------------
            with tc.tile_pool(name="wpool", bufs=1) as wpool, \
                 tc.tile_pool(name="lnb", bufs=3) as ln_pool, \
                 tc.tile_pool(name="stats", bufs=4) as st_pool, \
                 tc.tile_pool(name="pps", bufs=6, space="PSUM") as proj_ps, \
                 tc.tile_pool(name="tps", bufs=2, space="PSUM") as tp_ps:

                w_sb = {}
                for nm, dram in (("q", wq_d), ("k", wk_d), ("v", wv_d)):
                    w_sb[nm] = wpool.tile([128, KT, JW], BF16, name=f"w_{nm}")
                x1sb = wpool.tile([128, KT, n_seq], BF16, name="x1sb")
                x2sb = wpool.tile([128, KT, n_seq], BF16, name="x2sb")
                # DMA order matters: the first q matmul chain needs w_q and
                # the first x1 chunk only, so those go first
                xq = n_seq // 4
                x1r = x1t.rearrange("(kt p) n -> p kt n", p=128)
                x2r = x2t.rearrange("(kt p) n -> p kt n", p=128)
                # the first q matmul chain needs only w_q and x1 cols 0:128;
                # land those first so the PE starts ~5us in
                nc.sync.dma_start(
                    out=w_sb["q"],
                    in_=wq_d.rearrange("(kt p) j -> p kt j", p=128))
                nc.sync.dma_start(out=x1sb[:, :, 0:128], in_=x1r[:, :, 0:128])
                nc.sync.dma_start(
                    out=w_sb["k"],
                    in_=wk_d.rearrange("(kt p) j -> p kt j", p=128))
                nc.sync.dma_start(out=x2sb[:, :, 0:128], in_=x2r[:, :, 0:128])
                nc.sync.dma_start(
                    out=w_sb["v"],
                    in_=wv_d.rearrange("(kt p) j -> p kt j", p=128))
                nc.sync.dma_start(out=x1sb[:, :, 128:xq],
                                  in_=x1r[:, :, 128:xq])
                nc.sync.dma_start(out=x2sb[:, :, 128:xq],
                                  in_=x2r[:, :, 128:xq])
                for xi in range(1, 4):
                    xs = slice(xi * xq, (xi + 1) * xq)
                    nc.sync.dma_start(out=x1sb[:, :, xs], in_=x1r[:, :, xs])
                    nc.sync.dma_start(out=x2sb[:, :, xs], in_=x2r[:, :, xs])

                def layernorm_into(psum, dst, bias_b, gb, bb_, use_act):
                    # per-head LN of a [128, 512] projection tile
                    if bias_b is not None:
                        src = ln_pool.tile([128, JW], F32, name="biased",
                                           tag="biased")
                        nc.vector.tensor_add(out=src, in0=psum, in1=bias_b)
                    else:
                        src = psum
                    stats = st_pool.tile([128, HG, 6], F32, name="stats")
                    for h in range(HG):
                        nc.vector.bn_stats(
                            out=stats[:, h, :],
                            in_=src[:, h * HDIM:(h + 1) * HDIM])
                    mv = st_pool.tile([128, HG, 2], F32, name="mv")
                    for h in range(HG):
                        nc.vector.bn_aggr(out=mv[:, h, :], in_=stats[:, h, :])
                    std = st_pool.tile([128, HG], F32, name="std")
                    nc.scalar.activation(
                        out=std, in_=mv[:, :, 1],
                        func=mybir.ActivationFunctionType.Sqrt,
                        bias=eps_sb, scale=1.0)
                    rstd = st_pool.tile([128, HG], F32, name="rstd")
                    nc.vector.reciprocal(out=rstd, in_=std)
                    # bias for the ACT apply: -mean * rstd
                    negmr = st_pool.tile([128, HG], F32, name="negmr")
                    nc.vector.tensor_mul(out=negmr, in0=mv[:, :, 0],
                                         in1=rstd)
                    nc.vector.tensor_scalar(
                        out=negmr, in0=negmr, scalar1=-1.0, scalar2=None,
                        op0=mybir.AluOpType.mult)
                    for h in range(HG):
                        if use_act:
                            # (q-mu)*rstd == q*rstd + (-mu*rstd), one ACT op
                            nc.scalar.activation(
                                out=dst[:, h * HDIM:(h + 1) * HDIM],
                                in_=src[:, h * HDIM:(h + 1) * HDIM],
                                func=mybir.ActivationFunctionType.Identity,
                                bias=negmr[:, h:h + 1], scale=rstd[:, h:h + 1])
                        else:
                            nc.vector.tensor_scalar(
                                out=dst[:, h * HDIM:(h + 1) * HDIM],
                                in0=src[:, h * HDIM:(h + 1) * HDIM],
                                scalar1=mv[:, h, 0:1],
                                scalar2=rstd[:, h:h + 1],
                                op0=mybir.AluOpType.subtract,
                                op1=mybir.AluOpType.mult)
                    if gb is not None:
                        nc.vector.tensor_mul(out=dst, in0=dst, in1=gb)
                        nc.vector.tensor_add(out=dst, in0=dst, in1=bb_)

                def emit_transposes(ln, dstT, nt):
                    nsl = slice(nt * 128, (nt + 1) * 128)
                    for jt in range(4):
                        tp = tp_ps.tile([128, 128], BF16, name="tp", tag="tp")
                        nc.tensor.transpose(
                            tp, ln[:, jt * 128:(jt + 1) * 128], eye_sb)
                        # split explicitly: nc.any routes all of these to the
                        # already-saturated ACT (the projection pacer)
                        if jt % 2 == 0:
                            nc.vector.tensor_copy(out=dstT[:, jt, nsl], in_=tp)
                        else:
                            nc.scalar.copy(out=dstT[:, jt, nsl], in_=tp)

                # transposes run one n-tile behind the matmuls so the PE
                # never waits on a just-computed LN result
                pending = []
                for nt in range(nt_n):
                    nsl = slice(nt * 128, (nt + 1) * 128)
                    x1c = x1sb[:, :, nsl]
                    x2c = x2sb[:, :, nsl]

                    for nm, xc, dstT, bias_b, gb, bb_ in (
                        ("q", x1c, qT, bqb, gqb, btqb),
                        ("k", x2c, kT, bkb, gkb, btkb),
                    ):
                        ps = proj_ps.tile([128, JW], F32, name="ps", tag="ps")
                        for ct in range(KT):
                            nc.tensor.matmul(
                                ps, xc[:, ct, :], w_sb[nm][:, ct, :],
                                start=(ct == 0), stop=(ct == KT - 1))
                        ln = ln_pool.tile([128, JW], BF16, name="ln", tag="ln")
                        layernorm_into(ps, ln, bias_b, gb, bb_, True)
                        pending.append((ln, dstT, nt))

                    ps = proj_ps.tile([128, JW], F32, name="ps", tag="ps")
                    for ct in range(KT):
                        nc.tensor.matmul(
                            ps, x2c[:, ct, :], w_sb["v"][:, ct, :],
                            start=(ct == 0), stop=(ct == KT - 1))
                    psg = ps.rearrange("p (h d) -> p h d", h=HG)
                    if bvb is not None:
                        nc.vector.tensor_add(
                            out=vA[:, nt, :, 0:HDIM], in0=psg,
                            in1=bvb.rearrange("p (h d) -> p h d", h=HG))
                    else:
                        nc.vector.tensor_copy(out=vA[:, nt, :, 0:HDIM], in_=psg)
                    while len(pending) > 2:
                        emit_transposes(*pending.pop(0))
                for args in pending:
                    emit_transposes(*args)

            # ---------------- attention ----------------
            with tc.tile_pool(name="sps", bufs=2, space="PSUM") as s_ps, \
                 tc.tile_pool(name="pvps", bufs=2, space="PSUM") as pv_ps, \
                 tc.tile_pool(name="psb", bufs=3) as p_pool, \
                 tc.tile_pool(name="nrm", bufs=3) as n_pool, \
                 tc.tile_pool(name="dsc", bufs=4, space="DRAM") as dram_pool:
                for h in range(HG):
                    pt, bp = divmod(h, 2)
                    prows = slice(bp * 64, (bp + 1) * 64)
                    kTh = kT[prows, pt, :]
                    qTh = qT[prows, pt, :]
                    for blk in range(nblk):
                        pvs = [pv_ps.tile([65, 512], F32, name=f"pv{c2}",
                                          tag="pv") for c2 in range(nch)]
                        # software pipeline: PV matmuls run two m-tiles behind
                        # the score matmuls, so the PE queue never stalls on
                        # ACT's exp (a stalled PE FIFO keeps HAM at 1.2 GHz)
                        LAG = 2
                        p_tiles = {}
                        for mi in range(nt_n + LAG):
                            if mi < nt_n:
                                s = s_ps.tile([128, sw], F32, name="s",
                                              tag="s", bufs=LAG + 1)
                                for c2 in range(nch):
                                    n0 = blk * sw + c2 * 512
                                    nc.tensor.matmul(
                                        s[:, c2 * 512:(c2 + 1) * 512],
                                        kTh[:, mi * 128:(mi + 1) * 128],
                                        qTh[:, n0:n0 + 512],
                                        start=True, stop=True)
                                p = p_pool.tile([128, sw], BF16, name="p",
                                                tag="p", bufs=LAG + 2)
                                nc.scalar.activation(
                                    out=p, in_=s,
                                    func=mybir.ActivationFunctionType.Exp,
                                    scale=float(scale))
                                p_tiles[mi] = p
                            if mi >= LAG:
                                m = mi - LAG
                                p = p_tiles.pop(m)
                                for c2 in range(nch):
                                    nc.tensor.matmul(
                                        pvs[c2], vA[:, m, h, :],
                                        p[:, c2 * 512:(c2 + 1) * 512],
                                        start=(m == 0), stop=(m == nt_n - 1))
                        # Drain numerators + denominator rows out of PSUM
                        # first (frees the pv banks for the next block), then
                        # one batched reciprocal per block: rows parked at
                        # partitions 0/32 (compute APs need 32-aligned bases).
                        # The partition-broadcast goes through a DRAM bounce
                        # (SBUF APs cannot have a zero partition step) so the
                        # PE never has to wait on this chain.
                        dens = n_pool.tile([64, 512], F32, name="dens",
                                           tag="dens")
                        nc.vector.memset(dens, 1.0)
                        osb0s = []
                        for c2 in range(nch):
                            osb0 = n_pool.tile([64, 512], F32, name="osb0",
                                               tag="osb0", bufs=4)
                            nc.vector.tensor_copy(out=osb0, in_=pvs[c2][0:64, :])
                            nc.scalar.copy(out=dens[32 * c2:32 * c2 + 1, :],
                                           in_=pvs[c2][64:65, :])
                            osb0s.append(osb0)
                        denr = n_pool.tile([64, 512], F32, name="denr",
                                           tag="denr")
                        nc.vector.reciprocal(out=denr, in_=dens)
                        for c2 in range(nch):
                            dscr = dram_pool.tile([512], F32, name="dscr",
                                                  tag="dscr")
                            nc.sync.dma_start(
                                out=dscr, in_=denr[32 * c2:32 * c2 + 1, :])
                            denb_sb = n_pool.tile([64, 512], F32,
                                                  name="denb_sb",
                                                  tag="denb_sb")
                            dscr_b = bass.AP(tensor=dscr.tensor,
                                             offset=dscr.offset,
                                             ap=[[0, 64]] + list(dscr.ap))
                            nc.sync.dma_start(out=denb_sb, in_=dscr_b)
                            osb = n_pool.tile([64, 512], F32, name="osb",
                                              tag="osb")
                            nc.vector.tensor_mul(
                                out=osb, in0=osb0s[c2], in1=denb_sb)
                            n0 = blk * sw + c2 * 512
                            nc.sync.dma_start(
                                out=out_d[h * HDIM:(h + 1) * HDIM,
                                          n0:n0 + 512],
                                in_=osb)
    split_multi_waits(nc)
    return nc


def shard_inputs(x1, x2, wq, bq, wkv, bkv, gamma_q, beta_q, gamma_k, beta_k,
                 flags, n_seq=NSEQ):
    has_bq, has_bkv, has_gbq, has_gbk = flags
    bf16 = ml_dtypes.bfloat16
    eye = np.eye(128, dtype=bf16)
    in_maps = []
    for core in range(NCORES):
        b, g = divmod(core, 2)
        jsl = slice(g * JW, (g + 1) * JW)
        m = {
            "x1t": np.ascontiguousarray(x1[b, :n_seq].T.astype(bf16)),
            "x2t": np.ascontiguousarray(x2[b, :n_seq].T.astype(bf16)),
            "wq": np.ascontiguousarray(wq[:, jsl].astype(bf16)),
            "wk": np.ascontiguousarray(wkv[:, jsl].astype(bf16)),
            "wv": np.ascontiguousarray(
                wkv[:, DIM + g * JW:DIM + (g + 1) * JW].astype(bf16)),
            "eye": eye,
        }
        if has_bq:
            m["bq"] = np.ascontiguousarray(bq[jsl])
        if has_bkv:
            m["bk"] = np.ascontiguousarray(bkv[jsl])
            m["bv"] = np.ascontiguousarray(bkv[DIM + g * JW:DIM + (g + 1) * JW])
        if has_gbq:
            m["gq"] = np.tile(gamma_q, HG).astype(np.float32)
            m["betq"] = np.tile(beta_q, HG).astype(np.float32)
        if has_gbk:
            m["gk"] = np.tile(gamma_k, HG).astype(np.float32)
            m["betk"] = np.tile(beta_k, HG).astype(np.float32)
        in_maps.append(m)
    return in_maps


def kernel(x1, x2, wq, bq, wkv, bkv, gamma_q, beta_q, gamma_k, beta_k):
    x1 = np.asarray(x1, dtype=np.float32)
    x2 = np.asarray(x2, dtype=np.float32)
    wq = np.asarray(wq, dtype=np.float32)
    bq = np.asarray(bq, dtype=np.float32)
    wkv = np.asarray(wkv, dtype=np.float32)
    bkv = np.asarray(bkv, dtype=np.float32)
    gamma_q = np.asarray(gamma_q, dtype=np.float32)
    beta_q = np.asarray(beta_q, dtype=np.float32)
    gamma_k = np.asarray(gamma_k, dtype=np.float32)
    beta_k = np.asarray(beta_k, dtype=np.float32)

    flags = (
        bool(np.any(bq)),
        bool(np.any(bkv)),
        not (np.all(gamma_q == 1.0) and np.all(beta_q == 0.0)),
        not (np.all(gamma_k == 1.0) and np.all(beta_k == 0.0)),
    )
    nc = build(NSEQ, *flags)
    in_maps = shard_inputs(x1, x2, wq, bq, wkv, bkv, gamma_q, beta_q,
                           gamma_k, beta_k, flags)
    trace = bool(int(os.environ.get("KERNEL_TRACE", "0")))
    res = run_bass_kernel_spmd(nc, in_maps, core_ids=list(range(NCORES)),
                               trace=trace)
    global LAST_RESULTS
    LAST_RESULTS = res
    out = np.empty((B, NSEQ, DIM), dtype=np.float32)
    for core in range(NCORES):
        b, g = divmod(core, 2)
        out[b, :, g * JW:(g + 1) * JW] = res.results[core]["outT"].T
    return out
